# revision 6
# baseline (speedup 1.0000x reference)
"""Multi-head attention TRN2 kernel v2, 8-core (batch x head-block) sharded.

Problem (hardcoded): x[2,2048,1024] f32, Wq/Wk/Wv[1024,1024], Wo[1024,1024],
16 heads, dh=64. Reference computes softmax(Q K^T)/sqrt(1024) @ V @ Wo with the
division AFTER softmax (folded here into Wo as a host-side 1/32 scale).

Sharding: core c handles batch b=c//4 and head block hb=c%4 (4 heads = 256 dims:
Wq/Wk/Wv column slice, Wo row slice). Each core emits a partial Y[2048,1024];
host sums the 4 partials per batch.

v2 vs baseline:
- all matmul operands bf16 (PSUM accumulation stays f32); inputs cast on host
- QK^T runs as two concurrent K=64 matmuls row-packed into the PE array via
  tile_position (head 2g in rows 0:64, head 2g+1 in rows 64:128) — no zero
  padding, half the QK wall time
- ACT does exp only; PSUM->SBUF copies ride DVE (+ACT only while it is idle)
- softmax denominators still via ones-columns in the V operand (rows 64:128 of
  pO), but norm does one Newton step (not two) and frees pO via a fast copy
- the first head-pair's QK+exp strips are interleaved into phase 1 so the
  ACT engine (the bottleneck: 128 exps of [128,1024]) starts ~10us in
- V-copies batched 4-heads-at-a-time with 3D APs; ones written once by a
  strided gpsimd memset instead of a 4MB DMA
"""

import numpy as np
import ml_dtypes

import concourse.tile as tile
from concourse import bacc, mybir
from concourse.bass_utils import run_bass_kernel_spmd

N_CORES = 8
B = 2
S = 2048          # tokens per batch (= per core)
D = 1024          # model dim
DH = 64           # head dim
HPC = 4           # heads per core
DL = HPC * DH     # 256 local output dims per core
NG = DL // 128    # 2 partition groups of local dims
NK = D // 128     # 8 k-strips for QKV contraction
NT = S // 128     # 16 token strips
VW = 128          # V block: cols 0:64 = V dims, cols 64:128 = ones (denoms)

F32 = mybir.dt.float32
BF16 = mybir.dt.bfloat16
NPBF16 = ml_dtypes.bfloat16
EXP = mybir.ActivationFunctionType.Exp
MULT = mybir.AluOpType.mult
ADD = mybir.AluOpType.add


def build_nc(repeat=1, hw_loop=0, level=4, packed=True,
             pv_first=False, copies_dve=False, n1024=False):
    # level 5: PV but no norm; level 6: PV w/ constant lhsT, no norm
    # level: 0 = QKV proj only, 1 = +QK, 2 = +exp, 3 = +PV/norm, 4 = full
    nc = bacc.Bacc("TRN2", target_bir_lowering=False, debug=False)
    xT = nc.declare_dram_parameter("xT", [D, S], BF16, isOutput=False)
    Wq = nc.declare_dram_parameter("Wq", [D, DL], BF16, isOutput=False)
    Wk = nc.declare_dram_parameter("Wk", [D, DL], BF16, isOutput=False)
    Wv = nc.declare_dram_parameter("Wv", [D, DL], BF16, isOutput=False)
    Wo = nc.declare_dram_parameter("Wo", [DL, D], BF16, isOutput=False)
    Yp = nc.declare_dram_parameter("Yp", [S, D], BF16, isOutput=True)

    with tile.TileContext(nc) as tc:
        with tc.tile_pool(name="singles", bufs=1) as singles:
            wq_sb = singles.tile([128, NK * NG * 128], BF16)
            wk_sb = singles.tile([128, NK * NG * 128], BF16)
            wv_sb = singles.tile([128, NK * NG * 128], BF16)
            wo_sb = singles.tile([128, NG * D], BF16)
            # dim-major Q/K: group g cols g*S.., head 2g dims in rows 0:64,
            # head 2g+1 dims in rows 64:128
            qt_sb = singles.tile([128, NG * S], BF16)
            kt_sb = singles.tile([128, NG * S], BF16)
            ot_sb = singles.tile([128, NG * S], BF16)
            # vaug block (j*HPC+h)*VW: [128 tok, 64 v-dims | 64 ones]
            vaug_sb = singles.tile([128, NT * HPC * VW], BF16)

            def body():
                # ---- weight + ones setup ----
                # K and Q weights ride the sync queue (first matmuls need
                # them); V and Wo ride the scalar queue in parallel.  One
                # big rearranged DMA per weight: w_sb col block k*DL holds
                # W[k*128:(k+1)*128, :] (g blocks are adjacent).
                for w_dram, w_sb, eng in ((Wk, wk_sb, nc.sync),
                                          (Wq, wq_sb, nc.scalar),
                                          (Wv, wv_sb, nc.scalar)):
                    eng.dma_start(
                        out=w_sb[:].rearrange("p (k d) -> p k d", d=DL),
                        in_=w_dram[:].rearrange("(k p) d -> p k d", p=128),
                    )
                nc.scalar.dma_start(
                    out=wo_sb[:].rearrange("p (g d) -> p g d", d=D),
                    in_=Wo[:].rearrange("(g p) d -> p g d", p=128),
                )
                # ones columns of every vaug block, one strided memset
                ones_ap = vaug_sb[:].rearrange(
                    "p (b w) -> p b w", w=VW)[:, :, DH:VW]
                nc.gpsimd.memset(ones_ap, 1.0)
                # preload the exp table set while weights stream in
                nc.vector.memset(ot_sb[:, 0:4], 0.0)
                nc.scalar.activation(ot_sb[:, 4:8], ot_sb[:, 0:4], EXP)

                # ---- pools ----
                copy_engines = (nc.scalar, nc.vector)

                def copy_on(ci, dst, src):
                    eng = copy_engines[1 if copies_dve else (ci % 2)]
                    if eng is nc.scalar:
                        eng.copy(out=dst, in_=src)
                    else:
                        eng.tensor_copy(out=dst, in_=src)

                # phase2 QK+exp for one (pair g, query-half sh, key-strip j)
                def emit_qk_exp(g, sh, j, pSp, expp):
                    es = []
                    for rowoff in ((0, 64) if packed else (0, 64)):
                        if level == 7:
                            es.append(qt_sb[:, 0:1024])
                            continue
                        pS = pSp.tile([128, 1024], F32, name="pS")
                        lo, hi = (rowoff, rowoff + 64) if packed else (0, 128)
                        kts = kt_sb[lo:hi,
                                    g * S + j * 128:g * S + (j + 1) * 128]
                        if n1024:
                            cq = g * S + sh * 1024
                            nc.tensor.matmul(
                                pS[:], kts, qt_sb[lo:hi, cq:cq + 1024])
                        else:
                            for sc in range(2):
                                cq = g * S + sh * 1024 + sc * 512
                                nc.tensor.matmul(
                                    pS[:, sc * 512:(sc + 1) * 512],
                                    kts,
                                    qt_sb[lo:hi, cq:cq + 512],
                                )
                        if level >= 2:
                            e = expp.tile([128, 1024], BF16, name="expst")
                            nc.scalar.activation(e[:], pS[:], EXP)
                            es.append(e)
                        else:
                            es.append(None)
                    return es

                def emit_pv(g, sh, j, es, pO_pair):
                    for hh, (e, pO) in enumerate(zip(es, pO_pair)):
                        h = 2 * g + hh
                        vb = (h * VW) if level == 6 else ((j * HPC + h) * VW)
                        if n1024:
                            nc.tensor.matmul(
                                pO[:], vaug_sb[:, vb:vb + VW], e[:],
                                start=(j == 0), stop=True,
                                skip_group_check=True)
                        else:
                            for sc in range(2):
                                nc.tensor.matmul(
                                    pO[:, sc * 512:(sc + 1) * 512],
                                    vaug_sb[:, vb:vb + VW],
                                    e[:, sc * 512:(sc + 1) * 512],
                                    start=(j == 0),
                                    stop=True,
                                    skip_group_check=True,
                                )

                def emit_norm(g, sh, pO_pair, normp):
                    # pO rows 0:64 = PV sums, rows 64:128 = denominators.
                    # Copy sums to SBUF and read den straight from PSUM (a
                    # PSUM input may cross partitions; SB+SB inputs can't),
                    # so pO frees after the 2nd op and the next pair's PV
                    # doesn't stall.
                    for hh, pO in enumerate(pO_pair):
                        h = 2 * g + hh
                        pc = normp.tile([64, 1024], F32, name="pc")
                        nc.vector.tensor_copy(out=pc[:], in_=pO[0:DH, :])
                        rb = normp.tile([64, 1024], F32, name="rb")
                        e = normp.tile([64, 1024], F32, name="err")
                        den = pO[DH:128, :]
                        nc.vector.reciprocal(rb[:], den)
                        # DVE reciprocal is ~2^-5 accurate; 1 Newton step
                        nc.vector.tensor_tensor(out=e[:], in0=den, in1=rb[:],
                                                op=MULT)
                        nc.vector.tensor_scalar(e[:], e[:], -1.0, 2.0, MULT,
                                                ADD)
                        nc.vector.tensor_tensor(out=rb[:], in0=rb[:],
                                                in1=e[:], op=MULT)
                        r = (h % 2) * DH
                        nc.vector.tensor_tensor(
                            out=ot_sb[r:r + DH,
                                      g * S + sh * 1024:
                                      g * S + (sh + 1) * 1024],
                            in0=pc[:],
                            in1=rb[:],
                            op=MULT,
                        )

                # ---- phase 1 with interleaved (g0,sh0)+(g1,sh0) QK/exp ----
                # lite strips computed during phase 1; their PV runs right
                # after phase-1 pools close.  Pairs in (g, sh) order:
                # (0,0), (1,0), (0,1), (1,1) so phase-3 t<8 can start after
                # the second pair's norm.
                lite = {(0, 0): [], (1, 0): []}
                # eligibility: strip (g,sh=0,j) needs kt group g strips from
                # chunk c=j//4 and qt group g cols 0:1024 (chunks 0 and 1)
                # PSUM budget: pS 2x4KB + (phase1: pp1 4KB + ppv 2KB |
                # phase2: pO/pY 2x4KB) <= 16KB/partition
                lite_order = [(j, g) for j in range(8) for g in range(NG)]
                with tc.tile_pool(name="pS", bufs=2, space="PSUM") as pSp, \
                     tc.tile_pool(name="expp", bufs=36) as expp:
                    with tc.tile_pool(name="pp1", bufs=2, space="PSUM") as pp1, \
                         tc.tile_pool(name="ppv", bufs=2, space="PSUM") as ppv, \
                         tc.tile_pool(name="xkp", bufs=4) as xkp:
                        ci = 0
                        lq = list(lite_order)

                        def pop_lite(n=1):
                            if level < 1:
                                return
                            for _ in range(n):
                                if not lq:
                                    return
                                j, g = lq.pop(0)
                                lite[(g, 0)].append(
                                    emit_qk_exp(g, 0, j, pSp, expp))
                        xcs = []
                        for c in range(S // 512):
                            xc = xkp.tile([128, NK * 512], BF16, name="xc")
                            eng = nc.sync if c % 2 == 0 else nc.scalar
                            eng.dma_start(
                                out=xc[:].rearrange("p (k d) -> p k d", d=512),
                                in_=xT[:, c * 512:(c + 1) * 512].rearrange(
                                    "(k p) d -> p k d", p=128),
                            )
                            xcs.append(xc)
                            xks = [xc[:, k * 512:(k + 1) * 512]
                                   for k in range(NK)]
                            for w_sb, o_sb in ((wk_sb, kt_sb), (wq_sb, qt_sb)):
                                for g in range(NG):
                                    ps = pp1.tile([128, 512], F32,
                                                  name="ps_qk")
                                    for k in range(NK):
                                        cb = (k * NG + g) * 128
                                        nc.tensor.matmul(
                                            ps[:],
                                            w_sb[:, cb:cb + 128],
                                            xks[k],
                                            start=(k == 0),
                                            stop=(k == NK - 1),
                                        )
                                    cs = slice(c * 512, (c + 1) * 512)
                                    copy_on(ci, o_sb[:, g * S + cs.start:
                                                     g * S + cs.stop], ps[:])
                                    ci += 1
                                    if c >= 2:
                                        pop_lite()
                        
                        # phase 1b: V projections for all chunks, with the
                        # remaining lite strips interleaved — scores for the
                        # first pairs exist already, so ACT streams exps
                        # while the PE projects V
                        for c in range(S // 512):
                            xks = [xcs[c][:, k * 512:(k + 1) * 512]
                                   for k in range(NK)]
                            for t in range(4):
                                j = c * 4 + t
                                pv = ppv.tile([128, DL], F32, name="pv")
                                for k in range(NK):
                                    nc.tensor.matmul(
                                        pv[:],
                                        xks[k][:, t * 128:(t + 1) * 128],
                                        wv_sb[:, k * DL:(k + 1) * DL],
                                        start=(k == 0),
                                        stop=(k == NK - 1),
                                    )
                                # batched 4-head copy into vaug v-columns
                                vb = j * HPC * VW
                                dst = vaug_sb[:, vb:vb + HPC * VW].rearrange(
                                    "p (h w) -> p h w", w=VW)[:, :, 0:DH]
                                src = pv[:].rearrange("p (h d) -> p h d",
                                                      d=DH)
                                nc.vector.tensor_copy(out=dst, in_=src)
                                pop_lite()
                        pop_lite(len(lq))

                    # ---- phase 2 steady state (phase-1 PSUM pools closed) ----
                    # phase-3 pY tiles borrow the pO pool's two 4KB slots
                    with tc.tile_pool(name="pO", bufs=2, space="PSUM") as pOp, \
                         tc.tile_pool(name="normp", bufs=3) as normp, \
                         tc.tile_pool(name="ysbp", bufs=4) as ysbp:
                        p3cnt = 0

                        def emit_phase3(trange):
                            nonlocal p3cnt
                            for t in trange:
                                ysb = ysbp.tile([128, 1024], BF16,
                                                name="ysb")
                                for e2 in range(2):
                                    pY = pOp.tile([128, 512], F32, name="pO")
                                    for g in range(NG):
                                        nc.tensor.matmul(
                                            pY[:],
                                            ot_sb[:, g * S + t * 128:
                                                  g * S + (t + 1) * 128],
                                            wo_sb[:, g * D + e2 * 512:
                                                  g * D + (e2 + 1) * 512],
                                            start=(g == 0),
                                            stop=(g == NG - 1),
                                        )
                                    nc.vector.tensor_copy(
                                        out=ysb[:, e2 * 512:(e2 + 1) * 512],
                                        in_=pY[:])
                                    p3cnt += 1
                                eng = nc.sync if t % 2 else nc.scalar
                                eng.dma_start(
                                    out=Yp[t * 128:(t + 1) * 128, :],
                                    in_=ysb[:],
                                )

                        pairs = (((0, 0), (1, 0), (0, 1), (1, 1))
                                 if level >= 1 else ())
                        for pi, (g, sh) in enumerate(pairs):
                            lv3 = level >= 3
                            if lv3:
                                pO_pair = (
                                    pOp.tile([VW, 1024], F32, name="pO"),
                                    pOp.tile([VW, 1024], F32, name="pO"))
                            done = lite.get((g, sh), [])
                            if lv3:
                                for j, es in enumerate(done):
                                    emit_pv(g, sh, j, es, pO_pair)
                            prev = None
                            for j in range(len(done), NT):
                                if pv_first and prev is not None and lv3:
                                    emit_pv(g, sh, prev[0], prev[1], pO_pair)
                                es = emit_qk_exp(g, sh, j, pSp, expp)
                                if (not pv_first) and prev is not None and lv3:
                                    emit_pv(g, sh, prev[0], prev[1], pO_pair)
                                prev = (j, es)
                            if prev is not None and lv3:
                                emit_pv(g, sh, prev[0], prev[1], pO_pair)
                            if level in (3, 4):
                                emit_norm(g, sh, pO_pair, normp)
                            if level == 4:
                                if pi == 1:
                                    emit_phase3(range(0, 8))
                                elif pi == 3:
                                    emit_phase3(range(8, NT))

            if hw_loop:
                with tc.For_i(0, hw_loop, 1):
                    body()
            else:
                for _ in range(repeat):
                    body()
    nc.finalize()
    return nc


def make_in_maps(x, Wq, Wk, Wv, Wo):
    f = np.float32
    x = np.asarray(x, f)
    Wq, Wk, Wv, Wo = (np.asarray(a, f) for a in (Wq, Wk, Wv, Wo))
    in_maps = []
    xTs = [np.ascontiguousarray(x[b].T).astype(NPBF16) for b in range(B)]
    for c in range(N_CORES):
        b, hb = divmod(c, N_CORES // B)
        cols = slice(hb * DL, (hb + 1) * DL)
        in_maps.append({
            "xT": xTs[b],
            "Wq": np.ascontiguousarray(Wq[:, cols]).astype(NPBF16),
            "Wk": np.ascontiguousarray(Wk[:, cols]).astype(NPBF16),
            "Wv": np.ascontiguousarray(Wv[:, cols]).astype(NPBF16),
            "Wo": (np.ascontiguousarray(Wo[cols, :]) * f(1.0 / 32.0)
                   ).astype(NPBF16),
        })
    return in_maps


def run(inputs, trace=False, repeat=1, hw_loop=0):
    nc = build_nc(repeat=repeat, hw_loop=hw_loop)
    in_maps = make_in_maps(**inputs)
    res = run_bass_kernel_spmd(nc, in_maps, list(range(N_CORES)), trace=trace)
    yps = [np.asarray(res.results[c]["Yp"], np.float32)
           for c in range(N_CORES)]
    out = np.empty((B, S, D), np.float32)
    cpb = N_CORES // B
    for b in range(B):
        out[b] = sum(yps[b * cpb:(b + 1) * cpb])
    return out, res


def kernel(**inputs):
    out, _ = run(inputs, trace=False)
    return out


# revision 7
# speedup vs baseline: 1.1532x; 1.1532x over previous
"""Multi-head attention TRN2 kernel v2, 8-core (batch x head-block) sharded.

Problem (hardcoded): x[2,2048,1024] f32, Wq/Wk/Wv[1024,1024], Wo[1024,1024],
16 heads, dh=64. Reference computes softmax(Q K^T)/sqrt(1024) @ V @ Wo with the
division AFTER softmax (folded here into Wo as a host-side 1/32 scale).

Sharding: core c handles batch b=c//4 and head block hb=c%4 (4 heads = 256 dims:
Wq/Wk/Wv column slice, Wo row slice). Each core emits a partial Y[2048,1024];
host sums the 4 partials per batch.

v2 vs baseline:
- all matmul operands bf16 (PSUM accumulation stays f32); inputs cast on host
- QK^T runs as two concurrent K=64 matmuls row-packed into the PE array via
  tile_position (head 2g in rows 0:64, head 2g+1 in rows 64:128) — no zero
  padding, half the QK wall time
- ACT does exp only; PSUM->SBUF copies ride DVE (+ACT only while it is idle)
- softmax denominators still via ones-columns in the V operand (rows 64:128 of
  pO), but norm does one Newton step (not two) and frees pO via a fast copy
- the first head-pair's QK+exp strips are interleaved into phase 1 so the
  ACT engine (the bottleneck: 128 exps of [128,1024]) starts ~10us in
- V-copies batched 4-heads-at-a-time with 3D APs; ones written once by a
  strided gpsimd memset instead of a 4MB DMA
"""

import numpy as np
import ml_dtypes

import concourse.tile as tile
from concourse import bacc, mybir
from concourse.bass_utils import run_bass_kernel_spmd

N_CORES = 8
B = 2
S = 2048          # tokens per batch (= per core)
D = 1024          # model dim
DH = 64           # head dim
HPC = 4           # heads per core
DL = HPC * DH     # 256 local output dims per core
NG = DL // 128    # 2 partition groups of local dims
NK = D // 128     # 8 k-strips for QKV contraction
NT = S // 128     # 16 token strips
VW = 128          # V block: cols 0:64 = V dims, cols 64:128 = ones (denoms)

F32 = mybir.dt.float32
BF16 = mybir.dt.bfloat16
NPBF16 = ml_dtypes.bfloat16
EXP = mybir.ActivationFunctionType.Exp
MULT = mybir.AluOpType.mult
ADD = mybir.AluOpType.add


def build_nc(repeat=1, hw_loop=0, level=4, packed=True,
             pv_first=False, copies_dve=False, n1024=False,
             q4=False, hs=False, normpsum=True):
    # level 5: PV but no norm; level 6: PV w/ constant lhsT, no norm
    # level: 0 = QKV proj only, 1 = +QK, 2 = +exp, 3 = +PV/norm, 4 = full
    nc = bacc.Bacc("TRN2", target_bir_lowering=False, debug=False)
    xT = nc.declare_dram_parameter("xT", [D, S], BF16, isOutput=False)
    Wq = nc.declare_dram_parameter("Wq", [D, DL], BF16, isOutput=False)
    Wk = nc.declare_dram_parameter("Wk", [D, DL], BF16, isOutput=False)
    Wv = nc.declare_dram_parameter("Wv", [D, DL], BF16, isOutput=False)
    Wo = nc.declare_dram_parameter("Wo", [DL, D], BF16, isOutput=False)
    Yp = nc.declare_dram_parameter("Yp", [S, D], BF16, isOutput=True)

    with tile.TileContext(nc) as tc:
        with tc.tile_pool(name="singles", bufs=1) as singles:
            wq_sb = singles.tile([128, NK * NG * 128], BF16)
            wk_sb = singles.tile([128, NK * NG * 128], BF16)
            wv_sb = singles.tile([128, NK * NG * 128], BF16)
            wo_sb = singles.tile([128, NG * D], BF16)
            # dim-major Q/K: group g cols g*S.., head 2g dims in rows 0:64,
            # head 2g+1 dims in rows 64:128
            qt_sb = singles.tile([128, NG * S], BF16)
            kt_sb = singles.tile([128, NG * S], BF16)
            ot_sb = singles.tile([128, NG * S], BF16)
            # vaug block (j*HPC+h)*VW: [128 tok, 64 v-dims | 64 ones]
            vaug_sb = singles.tile([128, NT * HPC * VW], BF16)

            def body():
                # ---- weight + ones setup ----
                # K and Q weights ride the sync queue (first matmuls need
                # them); V and Wo ride the scalar queue in parallel.  One
                # big rearranged DMA per weight: w_sb col block k*DL holds
                # W[k*128:(k+1)*128, :] (g blocks are adjacent).
                for w_dram, w_sb, eng in ((Wk, wk_sb, nc.sync),
                                          (Wq, wq_sb, nc.scalar),
                                          (Wv, wv_sb, nc.scalar)):
                    eng.dma_start(
                        out=w_sb[:].rearrange("p (k d) -> p k d", d=DL),
                        in_=w_dram[:].rearrange("(k p) d -> p k d", p=128),
                    )
                nc.scalar.dma_start(
                    out=wo_sb[:].rearrange("p (g d) -> p g d", d=D),
                    in_=Wo[:].rearrange("(g p) d -> p g d", p=128),
                )
                # ones columns of every vaug block, one strided memset
                ones_ap = vaug_sb[:].rearrange(
                    "p (b w) -> p b w", w=VW)[:, :, DH:VW]
                nc.gpsimd.memset(ones_ap, 1.0)
                # preload the exp table set while weights stream in
                nc.vector.memset(ot_sb[:, 0:4], 0.0)
                nc.scalar.activation(ot_sb[:, 4:8], ot_sb[:, 0:4], EXP)

                # ---- pools ----
                copy_engines = (nc.scalar, nc.vector)

                def copy_on(ci, dst, src):
                    eng = copy_engines[1 if copies_dve else (ci % 2)]
                    if eng is nc.scalar:
                        eng.copy(out=dst, in_=src)
                    else:
                        eng.tensor_copy(out=dst, in_=src)

                # phase2 QK+exp for one (pair g, query-half sh, key-strip j)
                def emit_qk_exp(g, sh, j, pSp, expp):
                    es = []
                    for rowoff in ((0, 64) if packed else (0, 64)):
                        if level == 7:
                            es.append(qt_sb[:, 0:1024])
                            continue
                        pS = pSp.tile([128, 1024], F32, name="pS")
                        lo, hi = (rowoff, rowoff + 64) if packed else (0, 128)
                        kts = kt_sb[lo:hi,
                                    g * S + j * 128:g * S + (j + 1) * 128]
                        if n1024:
                            cq = g * S + sh * 1024
                            nc.tensor.matmul(
                                pS[:], kts, qt_sb[lo:hi, cq:cq + 1024])
                        else:
                            for sc in range(2):
                                cq = g * S + sh * 1024 + sc * 512
                                nc.tensor.matmul(
                                    pS[:, sc * 512:(sc + 1) * 512],
                                    kts,
                                    qt_sb[lo:hi, cq:cq + 512],
                                )
                        if level >= 2:
                            e = expp.tile([128, 1024], BF16, name="expst")
                            nc.scalar.activation(e[:], pS[:], EXP)
                            es.append(e)
                        else:
                            es.append(None)
                    return es

                def emit_pv(g, sh, j, es, pO_pair):
                    for hh, (e, pO) in enumerate(zip(es, pO_pair)):
                        h = 2 * g + hh
                        vb = (h * VW) if level == 6 else ((j * HPC + h) * VW)
                        if n1024:
                            nc.tensor.matmul(
                                pO[:], vaug_sb[:, vb:vb + VW], e[:],
                                start=(j == 0), stop=True,
                                skip_group_check=True)
                        else:
                            for sc in range(2):
                                nc.tensor.matmul(
                                    pO[:, sc * 512:(sc + 1) * 512],
                                    vaug_sb[:, vb:vb + VW],
                                    e[:, sc * 512:(sc + 1) * 512],
                                    start=(j == 0),
                                    stop=True,
                                    skip_group_check=True,
                                )

                def emit_qk_exp_hs(g, h, sh, j, pSp, expp):
                    ro = (h % 2) * 64
                    pS = pSp.tile([128, 1024], F32, name="pS")
                    kts = kt_sb[ro:ro + 64,
                                g * S + j * 128:g * S + (j + 1) * 128]
                    for sc in range(2):
                        cq = g * S + sh * 1024 + sc * 512
                        nc.tensor.matmul(
                            pS[:, sc * 512:(sc + 1) * 512],
                            kts,
                            qt_sb[ro:ro + 64, cq:cq + 512],
                        )
                    e = expp.tile([128, 1024], BF16, name="expst")
                    nc.scalar.activation(e[:], pS[:], EXP)
                    return e

                def emit_pv_hs(g, h, j, e, pOq, qt):
                    vb = (j * HPC + h) * VW
                    nc.tensor.matmul(
                        pOq[:],
                        vaug_sb[:, vb:vb + VW],
                        e[:, qt * 512:(qt + 1) * 512],
                        start=(j == 0),
                        stop=True,
                        skip_group_check=True,
                    )

                def emit_norm_hs(g, h, sh, qt, pOq, normp):
                    pc = normp.tile([64, 512], F32, name="pc")
                    nc.vector.tensor_copy(out=pc[:], in_=pOq[0:DH, :])
                    rb = normp.tile([64, 512], F32, name="rb")
                    e = normp.tile([64, 512], F32, name="err")
                    den = pOq[DH:128, :]
                    nc.vector.reciprocal(rb[:], den)
                    nc.vector.tensor_tensor(out=e[:], in0=den, in1=rb[:],
                                            op=MULT)
                    nc.vector.tensor_scalar(e[:], e[:], -1.0, 2.0, MULT, ADD)
                    nc.vector.tensor_tensor(out=rb[:], in0=rb[:], in1=e[:],
                                            op=MULT)
                    r = (h % 2) * DH
                    cb = g * S + sh * 1024 + qt * 512
                    nc.vector.tensor_tensor(
                        out=ot_sb[r:r + DH, cb:cb + 512],
                        in0=pc[:],
                        in1=rb[:],
                        op=MULT,
                    )

                def emit_pv_q(g, sh, j, es, pO_pair, qt):
                    for hh, (e, pO) in enumerate(zip(es, pO_pair)):
                        h = 2 * g + hh
                        vb = (j * HPC + h) * VW
                        nc.tensor.matmul(
                            pO[:],
                            vaug_sb[:, vb:vb + VW],
                            e[:, qt * 512:(qt + 1) * 512],
                            start=(j == 0),
                            stop=True,
                            skip_group_check=True,
                        )

                def emit_norm_q(g, sh, qt, pO_pair, normp):
                    for hh, pO in enumerate(pO_pair):
                        h = 2 * g + hh
                        pc = normp.tile([64, 512], F32, name="pc")
                        nc.vector.tensor_copy(out=pc[:], in_=pO[0:DH, :])
                        rb = normp.tile([64, 512], F32, name="rb")
                        e = normp.tile([64, 512], F32, name="err")
                        den = pO[DH:128, :]
                        nc.vector.reciprocal(rb[:], den)
                        nc.vector.tensor_tensor(out=e[:], in0=den, in1=rb[:],
                                                op=MULT)
                        nc.vector.tensor_scalar(e[:], e[:], -1.0, 2.0, MULT,
                                                ADD)
                        nc.vector.tensor_tensor(out=rb[:], in0=rb[:],
                                                in1=e[:], op=MULT)
                        r = (h % 2) * DH
                        cb = g * S + sh * 1024 + qt * 512
                        nc.vector.tensor_tensor(
                            out=ot_sb[r:r + DH, cb:cb + 512],
                            in0=pc[:],
                            in1=rb[:],
                            op=MULT,
                        )

                def emit_norm(g, sh, pO_pair, normp):
                    # pO rows 0:64 = PV sums, rows 64:128 = denominators.
                    # normpsum: skip the sums copy and let the final multiply
                    # read pO from PSUM directly (one less DVE op per head,
                    # pO held ~2us longer)
                    for hh, pO in enumerate(pO_pair):
                        h = 2 * g + hh
                        if normpsum:
                            pc = pO[0:DH, :]
                        else:
                            pc = normp.tile([64, 1024], F32, name="pc")
                            nc.vector.tensor_copy(out=pc[:], in_=pO[0:DH, :])
                        rb = normp.tile([64, 1024], F32, name="rb")
                        e = normp.tile([64, 1024], F32, name="err")
                        den = pO[DH:128, :]
                        nc.vector.reciprocal(rb[:], den)
                        # DVE reciprocal is ~2^-5 accurate; 1 Newton step
                        nc.vector.tensor_tensor(out=e[:], in0=den, in1=rb[:],
                                                op=MULT)
                        nc.vector.tensor_scalar(e[:], e[:], -1.0, 2.0, MULT,
                                                ADD)
                        nc.vector.tensor_tensor(out=rb[:], in0=rb[:],
                                                in1=e[:], op=MULT)
                        r = (h % 2) * DH
                        nc.vector.tensor_tensor(
                            out=ot_sb[r:r + DH,
                                      g * S + sh * 1024:
                                      g * S + (sh + 1) * 1024],
                            in0=pc if normpsum else pc[:],
                            in1=rb[:],
                            op=MULT,
                        )

                # ---- phase 1 with interleaved (g0,sh0)+(g1,sh0) QK/exp ----
                # lite strips computed during phase 1; their PV runs right
                # after phase-1 pools close.  Pairs in (g, sh) order:
                # (0,0), (1,0), (0,1), (1,1) so phase-3 t<8 can start after
                # the second pair's norm.
                lite = {(0, 0): [], (1, 0): []}
                # eligibility: strip (g,sh=0,j) needs kt group g strips from
                # chunk c=j//4 and qt group g cols 0:1024 (chunks 0 and 1)
                # PSUM budget: pS 2x4KB + (phase1: pp1 4KB + ppv 2KB |
                # phase2: pO/pY 2x4KB) <= 16KB/partition
                if hs:
                    lite_order = [(j, g, h) for j in range(8)
                                  for g in range(NG) for h in range(2)]
                else:
                    lite_order = [(j, g) for j in range(8)
                                  for g in range(NG)]
                with tc.tile_pool(name="pS", bufs=(3 if (q4 or hs) else 2),
                                  space="PSUM") as pSp, \
                     tc.tile_pool(name="expp",
                                  bufs=(40 if q4 else 36)) as expp:
                    with tc.tile_pool(name="pp1", bufs=2, space="PSUM") as pp1, \
                         tc.tile_pool(name="xkp", bufs=4) as xkp:
                        ppv = pp1
                        ci = 0
                        lq = list(lite_order)

                        def pop_lite(n=1):
                            if level < 1:
                                return
                            for _ in range(n):
                                if not lq:
                                    return
                                if hs:
                                    j, g, h = lq.pop(0)
                                    lite.setdefault((g, h), []).append(
                                        emit_qk_exp_hs(g, h, 0, j, pSp,
                                                       expp))
                                else:
                                    j, g = lq.pop(0)
                                    lite[(g, 0)].append(
                                        emit_qk_exp(g, 0, j, pSp, expp))
                        xcs = []
                        for c in range(S // 512):
                            xc = xkp.tile([128, NK * 512], BF16, name="xc")
                            eng = nc.sync if c % 2 == 0 else nc.scalar
                            eng.dma_start(
                                out=xc[:].rearrange("p (k d) -> p k d", d=512),
                                in_=xT[:, c * 512:(c + 1) * 512].rearrange(
                                    "(k p) d -> p k d", p=128),
                            )
                            xcs.append(xc)
                            xks = [xc[:, k * 512:(k + 1) * 512]
                                   for k in range(NK)]
                            for w_sb, o_sb in ((wk_sb, kt_sb), (wq_sb, qt_sb)):
                                for g in range(NG):
                                    ps = pp1.tile([128, 512], F32,
                                                  name="ps_qk")
                                    for k in range(NK):
                                        cb = (k * NG + g) * 128
                                        nc.tensor.matmul(
                                            ps[:],
                                            w_sb[:, cb:cb + 128],
                                            xks[k],
                                            start=(k == 0),
                                            stop=(k == NK - 1),
                                        )
                                    cs = slice(c * 512, (c + 1) * 512)
                                    copy_on(ci, o_sb[:, g * S + cs.start:
                                                     g * S + cs.stop], ps[:])
                                    ci += 1
                                    if c >= 2:
                                        pop_lite()
                        
                        # phase 1b: V projections for all chunks, with the
                        # remaining lite strips interleaved — scores for the
                        # first pairs exist already, so ACT streams exps
                        # while the PE projects V
                        for c in range(S // 512):
                            xks = [xcs[c][:, k * 512:(k + 1) * 512]
                                   for k in range(NK)]
                            for t in range(4):
                                j = c * 4 + t
                                pv = ppv.tile([128, DL], F32,
                                               name="ps_qk"
                                               if (q4 or hs) else "pv")
                                for k in range(NK):
                                    nc.tensor.matmul(
                                        pv[:],
                                        xks[k][:, t * 128:(t + 1) * 128],
                                        wv_sb[:, k * DL:(k + 1) * DL],
                                        start=(k == 0),
                                        stop=(k == NK - 1),
                                    )
                                # batched 4-head copy into vaug v-columns
                                vb = j * HPC * VW
                                dst = vaug_sb[:, vb:vb + HPC * VW].rearrange(
                                    "p (h w) -> p h w", w=VW)[:, :, 0:DH]
                                src = pv[:].rearrange("p (h d) -> p h d",
                                                      d=DH)
                                nc.vector.tensor_copy(out=dst, in_=src)
                                pop_lite()
                        pop_lite(len(lq))

                    # ---- phase 2 steady state (phase-1 PSUM pools closed) ----
                    # phase-3 pY tiles borrow the pO pool's two 4KB slots
                    with tc.tile_pool(name="pO", bufs=2, space="PSUM") as pOp, \
                         tc.tile_pool(name="normp", bufs=3) as normp, \
                         tc.tile_pool(name="ysbp", bufs=4) as ysbp:
                        p3cnt = 0

                        def emit_phase3(trange):
                            nonlocal p3cnt
                            for t in trange:
                                ysb = ysbp.tile([128, 1024], BF16,
                                                name="ysb")
                                for e2 in range(2):
                                    pY = pOp.tile([128, 512], F32, name="pO")
                                    for g in range(NG):
                                        nc.tensor.matmul(
                                            pY[:],
                                            ot_sb[:, g * S + t * 128:
                                                  g * S + (t + 1) * 128],
                                            wo_sb[:, g * D + e2 * 512:
                                                  g * D + (e2 + 1) * 512],
                                            start=(g == 0),
                                            stop=(g == NG - 1),
                                        )
                                    nc.vector.tensor_copy(
                                        out=ysb[:, e2 * 512:(e2 + 1) * 512],
                                        in_=pY[:])
                                    p3cnt += 1
                                eng = nc.sync if t % 2 else nc.scalar
                                eng.dma_start(
                                    out=Yp[t * 128:(t + 1) * 128, :],
                                    in_=ysb[:],
                                )

                        pairs = (((0, 0), (1, 0), (0, 1), (1, 1))
                                 if level >= 1 else ())
                        if hs and level >= 3:
                            groups = [(g, h, sh) for sh in range(2)
                                      for g in range(NG) for h in range(2)]
                            LAG1 = 9  # quarter-1 PV lags this many strips
                            for pi, (g, h, sh) in enumerate(groups):
                                esl = (list(lite.get((g, h), []))
                                       if sh == 0 else [])
                                pOq0 = pOp.tile([VW, 512], F32, name="pO")
                                pOq1 = pOp.tile([VW, 512], F32, name="pO")
                                for j, e in enumerate(esl):
                                    emit_pv_hs(g, h, j, e, pOq0, 0)
                                    if j >= LAG1:
                                        emit_pv_hs(g, h, j - LAG1,
                                                   esl[j - LAG1], pOq1, 1)
                                for j in range(len(esl), NT):
                                    e = emit_qk_exp_hs(g, h, sh, j, pSp,
                                                       expp)
                                    esl.append(e)
                                    if j >= 1:
                                        emit_pv_hs(g, h, j - 1, esl[j - 1],
                                                   pOq0, 0)
                                    if j >= LAG1:
                                        emit_pv_hs(g, h, j - LAG1,
                                                   esl[j - LAG1], pOq1, 1)
                                emit_pv_hs(g, h, NT - 1, esl[NT - 1],
                                           pOq0, 0)
                                emit_norm_hs(g, h, sh, 0, pOq0, normp)
                                for j in range(NT - LAG1, NT):
                                    emit_pv_hs(g, h, j, esl[j], pOq1, 1)
                                emit_norm_hs(g, h, sh, 1, pOq1, normp)
                                if pi == 3:
                                    emit_phase3(range(0, 8))
                                elif pi == 7:
                                    emit_phase3(range(8, NT))
                            pairs = ()
                        if q4 and level >= 3:
                            for pi, (g, sh) in enumerate(pairs):
                                esl = list(lite.get((g, sh), []))
                                pOq0 = (pOp.tile([128, 512], F32, name="pO"),
                                        pOp.tile([128, 512], F32, name="pO"))
                                for j, es in enumerate(esl):
                                    emit_pv_q(g, sh, j, es, pOq0, 0)
                                prev = None
                                for j in range(len(esl), NT):
                                    es = emit_qk_exp(g, sh, j, pSp, expp)
                                    esl.append(es)
                                    if prev is not None:
                                        emit_pv_q(g, sh, prev[0], prev[1],
                                                  pOq0, 0)
                                    prev = (j, es)
                                if prev is not None:
                                    emit_pv_q(g, sh, prev[0], prev[1],
                                              pOq0, 0)
                                # pre-emit the next pair's first strips so
                                # ACT streams their exps during this pair's
                                # quarter-1 burst and norms
                                if pi + 1 < len(pairs):
                                    ng_, nsh = pairs[pi + 1]
                                    nl = lite.setdefault((ng_, nsh), [])
                                    for j in range(len(nl), min(len(nl) + 3,
                                                                NT)):
                                        nl.append(emit_qk_exp(ng_, nsh, j,
                                                              pSp, expp))
                                emit_norm_q(g, sh, 0, pOq0, normp)
                                pOq1 = (pOp.tile([128, 512], F32, name="pO"),
                                        pOp.tile([128, 512], F32, name="pO"))
                                for j, es in enumerate(esl):
                                    emit_pv_q(g, sh, j, es, pOq1, 1)
                                emit_norm_q(g, sh, 1, pOq1, normp)
                                if pi == 3:
                                    emit_phase3(range(0, NT))
                            pairs = ()
                        for pi, (g, sh) in enumerate(pairs):
                            lv3 = level >= 3
                            if lv3:
                                pO_pair = (
                                    pOp.tile([VW, 1024], F32, name="pO"),
                                    pOp.tile([VW, 1024], F32, name="pO"))
                            done = lite.get((g, sh), [])
                            if lv3:
                                for j, es in enumerate(done):
                                    emit_pv(g, sh, j, es, pO_pair)
                            prev = None
                            for j in range(len(done), NT):
                                if pv_first and prev is not None and lv3:
                                    emit_pv(g, sh, prev[0], prev[1], pO_pair)
                                es = emit_qk_exp(g, sh, j, pSp, expp)
                                if (not pv_first) and prev is not None and lv3:
                                    emit_pv(g, sh, prev[0], prev[1], pO_pair)
                                prev = (j, es)
                            if prev is not None and lv3:
                                emit_pv(g, sh, prev[0], prev[1], pO_pair)
                            if level in (3, 4):
                                emit_norm(g, sh, pO_pair, normp)
                            if level == 4:
                                if pi == 1:
                                    emit_phase3(range(0, 8))
                                elif pi == 3:
                                    emit_phase3(range(8, NT))

            if hw_loop:
                with tc.For_i(0, hw_loop, 1):
                    body()
            else:
                for _ in range(repeat):
                    body()
    nc.finalize()
    return nc


def make_in_maps(x, Wq, Wk, Wv, Wo):
    f = np.float32
    x = np.asarray(x, f)
    Wq, Wk, Wv, Wo = (np.asarray(a, f) for a in (Wq, Wk, Wv, Wo))
    in_maps = []
    xTs = [np.ascontiguousarray(x[b].T).astype(NPBF16) for b in range(B)]
    for c in range(N_CORES):
        b, hb = divmod(c, N_CORES // B)
        cols = slice(hb * DL, (hb + 1) * DL)
        in_maps.append({
            "xT": xTs[b],
            "Wq": np.ascontiguousarray(Wq[:, cols]).astype(NPBF16),
            "Wk": np.ascontiguousarray(Wk[:, cols]).astype(NPBF16),
            "Wv": np.ascontiguousarray(Wv[:, cols]).astype(NPBF16),
            "Wo": (np.ascontiguousarray(Wo[cols, :]) * f(1.0 / 32.0)
                   ).astype(NPBF16),
        })
    return in_maps


def run(inputs, trace=False, repeat=1, hw_loop=0):
    nc = build_nc(repeat=repeat, hw_loop=hw_loop)
    in_maps = make_in_maps(**inputs)
    res = run_bass_kernel_spmd(nc, in_maps, list(range(N_CORES)), trace=trace)
    yps = [np.asarray(res.results[c]["Yp"], np.float32)
           for c in range(N_CORES)]
    out = np.empty((B, S, D), np.float32)
    cpb = N_CORES // B
    for b in range(B):
        out[b] = sum(yps[b * cpb:(b + 1) * cpb])
    return out, res


def kernel(**inputs):
    out, _ = run(inputs, trace=False)
    return out
